# revision 1
# baseline (speedup 1.0000x reference)
"""Trainium2 Bass kernel for the BayesianSkipgram loss.

Strategy (8 NeuronCores, batch-sharded, no collectives):
  - Each core computes the per-sample loss for its 256-sample batch shard.
  - Encoder runs in "T layout" ([D partitions, sample free-dim]) so every
    matmul contraction lands on the partition axis with host-pretransposed
    weights (no on-device weight transposes).
  - The dominant [B, N] logits are never materialized: for each batch chunk
    of 128, logits stream through PSUM in 2048-column groups (4 banks), and
    ScalarE computes Exp with a fused per-partition accumulate (accum_out)
    giving sum(exp(logits)) directly. W_gen^T is uploaded pre-cast to bf16
    and held resident in SBUF (~98KB/partition).
  - take_along(logp) is computed exactly as z . W_gen[ctx] + b_gen[ctx]
    - C*logsumexp, via fp32 indirect-DMA row gathers (one per context
    position) dotted against z in natural layout. b_gen rides along as a
    129th column of W_gen so the bias is part of the same dot product.
  - Host combines the 8x[256] per-sample losses with a mean.
"""

import numpy as np
import ml_dtypes

import concourse.bass as bass
import concourse.mybir as mybir
import concourse.tile as tile
from concourse import bacc
from concourse.bass import IndirectOffsetOnAxis
from concourse.bass_utils import run_bass_kernel_spmd

F32 = mybir.dt.float32
BF16 = mybir.dt.bfloat16
I32 = mybir.dt.int32
AF = mybir.ActivationFunctionType
ALU = mybir.AluOpType

N = 50257      # vocab
D = 128        # embedding dim
B = 2048       # total batch
C = 10         # context size
NCORES = 8
BS = B // NCORES          # 256 samples per core
M = BS // 128             # 2 partition chunks of samples
NFLAT = BS * C            # 2560 flattened (sample, ctx) pairs per core
NT = NFLAT // 128         # 20 gather tiles
GROUP = 2048              # psum group (4 banks of fp32)
NGROUPS = (N + GROUP - 1) // GROUP   # 25
CHUNK = 512               # matmul free-dim (one psum bank)

_CACHE = {}


def _patch_act_tables():
    """Keep Exp/Ln/Identity/Copy only in natural_log_exp_and_others so the
    table-load inserter uses one set for the whole kernel (one ~2.7us load
    instead of thrashing between exp_and_others / natural_log)."""
    import concourse.bacc as _bacc_mod
    import concourse.hw_specs as _hws
    if getattr(_bacc_mod, "_ant_act_tables_patched", False):
        return
    _orig = _hws.get_activation_tables
    _ours = {AF.Exp, AF.Ln, AF.Identity, AF.Copy}

    def _filtered(arch):
        tabs = _orig(arch)
        out = {}
        for name, funcs in tabs.items():
            if name == "natural_log_exp_and_others" or not (_ours & funcs):
                out[name] = funcs
            else:
                out[name] = funcs - _ours
        return out

    _bacc_mod.get_activation_tables = _filtered
    _bacc_mod._ant_act_tables_patched = True


def _build(with_bgen: bool, stage: int = 99):
    """stage: dev-only truncation knob. 1=io, 2=+gathers, 3=+encoder/kl/
    takealong, 4=+bigloop, 99=full."""
    _patch_act_tables()
    nc = bacc.Bacc("TRN2", target_bir_lowering=False, debug=False)

    # ---------------- DRAM I/O ----------------
    d_ctx_idx = nc.dram_tensor("ctx_idx", [128, NT], I32, kind="ExternalInput")
    d_ctx_byc = nc.dram_tensor("ctx_byc", [128, M * C], I32, kind="ExternalInput")
    d_x_idx = nc.dram_tensor("x_idx", [128, M], I32, kind="ExternalInput")
    d_epsT = nc.dram_tensor("epsT", [128, BS], F32, kind="ExternalInput")
    d_waff1T = nc.dram_tensor("waff1T", [128, 128], F32, kind="ExternalInput")
    d_waff2T = nc.dram_tensor("waff2T", [128, 128], F32, kind="ExternalInput")
    d_wmuT = nc.dram_tensor("wmuT", [128, 128], F32, kind="ExternalInput")
    d_wsigT = nc.dram_tensor("wsigT", [128, 128], F32, kind="ExternalInput")
    d_baff = nc.dram_tensor("baff", [128, 1], F32, kind="ExternalInput")
    d_bmu = nc.dram_tensor("bmu", [128, 1], F32, kind="ExternalInput")
    d_bsig = nc.dram_tensor("bsig", [128, 1], F32, kind="ExternalInput")
    d_inf_emb = nc.dram_tensor("inf_emb", [N, D], F32, kind="ExternalInput")
    d_gsig_emb = nc.dram_tensor("gsig_emb", [N, D], F32, kind="ExternalInput")
    d_wg_aug = nc.dram_tensor("wg_aug", [N, D + 1], F32, kind="ExternalInput")
    d_wgT = nc.dram_tensor("wgT", [128, N], BF16, kind="ExternalInput")
    d_ident = nc.dram_tensor("ident", [128, 128], F32, kind="ExternalInput")
    if with_bgen:
        NCH = (N + CHUNK - 1) // CHUNK  # 99
        d_bgen = nc.dram_tensor("bgen2d", [NCH, CHUNK], BF16, kind="ExternalInput")
    d_loss = nc.dram_tensor("loss_part", [BS], F32, kind="ExternalOutput")

    with tile.TileContext(nc) as tc:
        cpool = tc.alloc_tile_pool(name="consts", bufs=1)
        wgpool = tc.alloc_tile_pool(name="wg", bufs=1)
        encpool = tc.alloc_tile_pool(name="enc", bufs=1)
        gpool = tc.alloc_tile_pool(name="gath", bufs=3)
        dpool = tc.alloc_tile_pool(name="dram", bufs=1, space="DRAM")

        # ---- constants / small inputs ----
        # identity comes from the host: make_identity would occupy GpSimd for
        # ~7us and delay the serial indirect-gather chain behind it
        identity = cpool.tile([128, 128], F32)
        nc.sync.dma_start(out=identity[:], in_=d_ident[:, :])
        ones_col = cpool.tile([128, 1], F32)
        nc.vector.memset(ones_col, 1.0)

        ctxi = cpool.tile([128, NT], I32)
        nc.sync.dma_start(out=ctxi[:], in_=d_ctx_idx[:, :])
        ctxbc = cpool.tile([128, M * C], I32)
        nc.sync.dma_start(out=ctxbc[:], in_=d_ctx_byc[:, :])
        xi = cpool.tile([128, M], I32)
        nc.sync.dma_start(out=xi[:], in_=d_x_idx[:, :])

        epsT = cpool.tile([128, BS], F32)
        nc.sync.dma_start(out=epsT[:], in_=d_epsT[:, :])
        waff1T = cpool.tile([128, 128], F32)
        nc.sync.dma_start(out=waff1T[:], in_=d_waff1T[:, :])
        waff2T = cpool.tile([128, 128], F32)
        nc.sync.dma_start(out=waff2T[:], in_=d_waff2T[:, :])
        wmuT = cpool.tile([128, 128], F32)
        nc.sync.dma_start(out=wmuT[:], in_=d_wmuT[:, :])
        wsigT = cpool.tile([128, 128], F32)
        nc.sync.dma_start(out=wsigT[:], in_=d_wsigT[:, :])
        baff = cpool.tile([128, 1], F32)
        nc.sync.dma_start(out=baff[:], in_=d_baff[:, :])
        bmu = cpool.tile([128, 1], F32)
        nc.sync.dma_start(out=bmu[:], in_=d_bmu[:, :])
        bsig = cpool.tile([128, 1], F32)
        nc.sync.dma_start(out=bsig[:], in_=d_bsig[:, :])

        # ---- resident W_gen^T (bf16), loaded in column blocks (emitted after
        # the small input DMAs so they don't queue behind 13MB of weights) ----
        wg_sb = wgpool.tile([128, N], BF16)
        if stage >= 4:
            WBLK = 4096
            for c0 in range(0, N, WBLK):
                c1 = min(c0 + WBLK, N)
                nc.sync.dma_start(out=wg_sb[:, c0:c1], in_=d_wgT[:, c0:c1])

        # persistent encoder tensors
        ctxT = encpool.tile([128, NFLAT], F32)
        centerT = encpool.tile([128, BS], F32)
        gsigT = encpool.tile([128, BS], F32)
        h_pre = encpool.tile([128, NFLAT], F32)
        h_sumT = encpool.tile([128, BS], F32)
        muT = encpool.tile([128, BS], F32)
        infsigT = encpool.tile([128, BS], F32)
        sigmaT = encpool.tile([128, BS], F32)
        zT = encpool.tile([128, BS], F32)
        z_bf = encpool.tile([128, BS], BF16)
        z_nat = encpool.tile([128, M * (D + 1)], F32)   # [z | 1] natural per m
        tal_bc = encpool.tile([128, M * C], F32)
        sums = encpool.tile([128, M * 32], F32)     # exp partial sums, col g/group
        talsum = encpool.tile([128, M], F32)
        lse = encpool.tile([128, M], F32)
        kl_rt = encpool.tile([128, M], F32)
        tot = encpool.tile([128, M], F32)
        loss_sb = encpool.tile([128, M], F32)

        kl_d = dpool.tile([BS], F32)

        if stage < 99:
            nc.vector.memset(loss_sb[:, :], float(stage))

        with tc.tile_pool(name="encps", bufs=2, space="PSUM") as encps, \
             tc.tile_pool(name="mmps", bufs=1, space="PSUM") as mmps:

            # ---- gathers + PE transposes ----
            # (one indirect DMA per 128 indices: multi-index offset APs pass
            # CoreSim but return garbage on hardware SWDGE)
            def gather_multi(src_dram, idx_ap, nat_tile, k):
                d = src_dram.shape[1]
                for t in range(k):
                    nc.gpsimd.indirect_dma_start(
                        out=nat_tile[:, t * d:(t + 1) * d],
                        out_offset=None,
                        in_=src_dram[:, :],
                        in_offset=IndirectOffsetOnAxis(
                            ap=idx_ap[:, t:t + 1], axis=0),
                    )

            def transpose_to(nat_tile, dstT, k):
                for t in range(k):
                    ps = encps.tile([128, 128], F32, tag="tps")
                    nc.tensor.transpose(out=ps[:, :],
                                        in_=nat_tile[:, t * 128:(t + 1) * 128],
                                        identity=identity[:, :])
                    nc.vector.tensor_copy(out=dstT[:, t * 128:(t + 1) * 128],
                                          in_=ps[:, :])

            if stage >= 2:
                ctx_nat = encpool.tile([128, NFLAT], F32)
                gather_multi(d_inf_emb, ctxi[:, :], ctx_nat, NT)
                cg_nat = encpool.tile([128, 2 * M * 128], F32)
                gather_multi(d_inf_emb, xi[:, :], cg_nat[:, :M * 128], M)
                gather_multi(d_gsig_emb, xi[:, :], cg_nat[:, M * 128:], M)
                transpose_to(ctx_nat, ctxT, NT)
                transpose_to(cg_nat[:, :M * 128], centerT, M)
                transpose_to(cg_nat[:, M * 128:], gsigT, M)

            if stage >= 3:
                # ---- encoder ----
                apre_ps = mmps.tile([128, BS], F32, tag="apre")
                nc.tensor.matmul(out=apre_ps[:, :], lhsT=waff1T[:, :],
                                 rhs=centerT[:, :], start=True, stop=True)
                apre = encpool.tile([128, BS], F32)
                nc.scalar.activation(out=apre[:, :], in_=apre_ps[:, :],
                                     func=AF.Identity, bias=baff[:, 0:1], scale=1.0)

                # Bpre in two b-aligned halves (1280 cols = 128 samples * C)
                HALF = NFLAT // 2
                for hh in range(2):
                    bpre_ps = mmps.tile([128, HALF], F32, tag="bpre")
                    r0 = hh * HALF
                    for j0 in range(0, HALF, CHUNK):
                        j1 = min(j0 + CHUNK, HALF)
                        nc.tensor.matmul(out=bpre_ps[:, j0:j1],
                                         lhsT=waff2T[:, :],
                                         rhs=ctxT[:, r0 + j0:r0 + j1],
                                         start=True, stop=True)
                    bpre3 = bpre_ps[:, :].rearrange("p (b c) -> p b c", c=C)
                    h3 = h_pre[:, r0:r0 + HALF].rearrange("p (b c) -> p b c", c=C)
                    nc.vector.tensor_tensor(
                        out=h3, in0=bpre3,
                        in1=apre[:, hh * 128:(hh + 1) * 128]
                        .to_broadcast([128, 128, C]),
                        op=ALU.add)
                nc.vector.tensor_scalar_max(out=h_pre[:, :], in0=h_pre[:, :],
                                            scalar1=0.0)
                nc.vector.reduce_sum(
                    out=h_sumT[:, :],
                    in_=h_pre[:, :].rearrange("p (b c) -> p b c", c=C),
                    axis=mybir.AxisListType.X)

                # mu / inf_sigma
                mu_ps = mmps.tile([128, BS], F32, tag="apre")
                nc.tensor.matmul(out=mu_ps[:, :], lhsT=wmuT[:, :], rhs=h_sumT[:, :],
                                 start=True, stop=True)
                nc.scalar.activation(out=muT[:, :], in_=mu_ps[:, :],
                                     func=AF.Identity, bias=bmu[:, 0:1], scale=1.0)

                sig_ps = mmps.tile([128, BS], F32, tag="apre")
                nc.tensor.matmul(out=sig_ps[:, :], lhsT=wsigT[:, :],
                                 rhs=h_sumT[:, :], start=True, stop=True)
                # softplus(x) = ln(1 + exp(x)) via Exp/Ln (one ACT table set)
                sp_e = encpool.tile([128, BS], F32)
                nc.scalar.activation(out=sp_e[:, :], in_=sig_ps[:, :],
                                     func=AF.Exp, bias=bsig[:, 0:1], scale=1.0)
                nc.vector.tensor_scalar_add(out=sp_e[:, :], in0=sp_e[:, :],
                                            scalar1=1.0)
                nc.scalar.activation(out=infsigT[:, :], in_=sp_e[:, :], func=AF.Ln)

                sp_g = encpool.tile([128, BS], F32)
                nc.scalar.activation(out=sp_g[:, :], in_=gsigT[:, :], func=AF.Exp)
                nc.vector.tensor_scalar_add(out=sp_g[:, :], in0=sp_g[:, :],
                                            scalar1=1.0)
                nc.scalar.activation(out=sigmaT[:, :], in_=sp_g[:, :], func=AF.Ln)

                # z = mu + eps * inf_sigma
                nc.vector.tensor_tensor(out=zT[:, :], in0=epsT[:, :],
                                        in1=infsigT[:, :], op=ALU.mult)
                nc.vector.tensor_tensor(out=zT[:, :], in0=zT[:, :], in1=muT[:, :],
                                        op=ALU.add)
                nc.vector.tensor_copy(out=z_bf[:, :], in_=zT[:, :])

                # ---- KL per-sample (partition-reduce via ones matmul) ----
                kli = encpool.tile([128, BS], F32)
                tmp = encpool.tile([128, BS], F32)
                nc.scalar.activation(out=kli[:, :], in_=sigmaT[:, :], func=AF.Ln)
                nc.scalar.activation(out=tmp[:, :], in_=infsigT[:, :], func=AF.Ln)
                nc.vector.tensor_tensor(out=kli[:, :], in0=kli[:, :], in1=tmp[:, :],
                                        op=ALU.subtract)
                num = encpool.tile([128, BS], F32)
                nc.vector.tensor_tensor(out=num[:, :], in0=muT[:, :],
                                        in1=sigmaT[:, :], op=ALU.subtract)
                nc.vector.tensor_tensor(out=num[:, :], in0=num[:, :], in1=num[:, :],
                                        op=ALU.mult)
                nc.vector.tensor_tensor(out=tmp[:, :], in0=infsigT[:, :],
                                        in1=infsigT[:, :], op=ALU.mult)
                nc.vector.tensor_tensor(out=num[:, :], in0=num[:, :], in1=tmp[:, :],
                                        op=ALU.add)
                nc.vector.tensor_tensor(out=tmp[:, :], in0=sigmaT[:, :],
                                        in1=sigmaT[:, :], op=ALU.mult)
                nc.vector.tensor_scalar_mul(out=tmp[:, :], in0=tmp[:, :],
                                            scalar1=2.0)
                nc.vector.reciprocal(out=tmp[:, :], in_=tmp[:, :])
                nc.vector.tensor_tensor(out=num[:, :], in0=num[:, :], in1=tmp[:, :],
                                        op=ALU.mult)
                nc.vector.tensor_tensor(out=kli[:, :], in0=kli[:, :], in1=num[:, :],
                                        op=ALU.add)
                nc.vector.tensor_scalar_add(out=kli[:, :], in0=kli[:, :],
                                            scalar1=-0.5)
                kl_ps = mmps.tile([1, BS], F32, tag="klps")
                nc.tensor.matmul(out=kl_ps[:, :], lhsT=ones_col[:, :],
                                 rhs=kli[:, :], start=True, stop=True)
                kl_row = encpool.tile([1, BS], F32)
                nc.vector.tensor_copy(out=kl_row[:, :], in_=kl_ps[:, :])
                nc.sync.dma_start(out=kl_d[:].rearrange("(a b) -> a b", a=1),
                                  in_=kl_row[:, :])
                nc.sync.dma_start(out=kl_rt[:, :],
                                  in_=kl_d[:].rearrange("(m p) -> p m", p=128))

                # ---- z in natural layout, augmented with a ones column ----
                for m in range(M):
                    zps = encps.tile([128, 128], F32, tag="tps")
                    nc.tensor.transpose(out=zps[:, :],
                                        in_=zT[:, m * 128:(m + 1) * 128],
                                        identity=identity[:, :])
                    a0 = m * (D + 1)
                    nc.vector.tensor_copy(out=z_nat[:, a0:a0 + D], in_=zps[:, :])
                    nc.vector.tensor_copy(out=z_nat[:, a0 + D:a0 + D + 1],
                                          in_=ones_col[:, :])

                # ---- take-along dots: tal_bc[p, m*C+c] =
                #      z_aug[m*128+p] . wg_aug[ctx[m*128+p, c]] ----
                dummy = encpool.tile([128, D + 1], F32)
                wrows = encpool.tile([128, M * C * (D + 1)], F32)
                gather_multi(d_wg_aug, ctxbc[:, :], wrows, M * C)
                for m in range(M):
                    a0 = m * (D + 1)
                    for c in range(C):
                        col = m * C + c
                        w0 = col * (D + 1)
                        nc.vector.tensor_tensor(
                            out=dummy[:, :], in0=wrows[:, w0:w0 + D + 1],
                            in1=z_nat[:, a0:a0 + D + 1], op=ALU.mult)
                        nc.vector.reduce_sum(out=tal_bc[:, col:col + 1],
                                             in_=dummy[:, :],
                                             axis=mybir.AxisListType.X)
                for m in range(M):
                    nc.vector.reduce_sum(out=talsum[:, m:m + 1],
                                         in_=tal_bc[:, m * C:(m + 1) * C],
                                         axis=mybir.AxisListType.X)

        # ---- the big streamed logits/exp loop ----
        if with_bgen and stage >= 4:
            ones_bf = cpool.tile([1, 128], BF16)
            nc.vector.memset(ones_bf, 1.0)
        with tc.tile_pool(name="bigps", bufs=2, space="PSUM") as bigps, \
             tc.tile_pool(name="expo", bufs=2) as expop, \
             tc.tile_pool(name="bgst", bufs=4) as bgstp:
            if stage >= 4:
                for m in range(M):
                    zcol = z_bf[:, m * 128:(m + 1) * 128]
                    for g in range(NGROUPS):
                        c0 = g * GROUP
                        gw = min(GROUP, N - c0)
                        ps = bigps.tile([128, GROUP], F32, tag="big")
                        for j0 in range(0, gw, CHUNK):
                            j1 = min(j0 + CHUNK, gw)
                            nc.tensor.matmul(
                                out=ps[:, j0:j1], lhsT=zcol,
                                rhs=wg_sb[:, c0 + j0:c0 + j1],
                                start=True, stop=not with_bgen,
                            )
                            if with_bgen:
                                bg = bgstp.tile([1, CHUNK], BF16, tag="bg")
                                nc.sync.dma_start(
                                    out=bg[:, :j1 - j0],
                                    in_=d_bgen[(c0 + j0) // CHUNK, :j1 - j0]
                                    .rearrange("(a b) -> a b", a=1))
                                nc.tensor.matmul(
                                    out=ps[:, j0:j1], lhsT=ones_bf[:, :],
                                    rhs=bg[:, :j1 - j0], start=False, stop=True,
                                )
                        eo = expop.tile([128, GROUP], BF16, tag="eo")
                        nc.scalar.activation(
                            out=eo[:, :gw], in_=ps[:, :gw], func=AF.Exp,
                            accum_out=sums[:, m * 32 + g:m * 32 + g + 1],
                        )

        # ---- epilogue: lse, loss assembly ----
        if stage >= 99:
            for m in range(M):
                nc.vector.reduce_sum(out=tot[:, m:m + 1],
                                     in_=sums[:, m * 32:m * 32 + NGROUPS],
                                     axis=mybir.AxisListType.X)
            nc.scalar.activation(out=lse[:, :], in_=tot[:, :], func=AF.Ln)
            # loss = kl - talong_sum + C * lse
            nc.vector.tensor_tensor(out=loss_sb[:, :], in0=kl_rt[:, :],
                                    in1=talsum[:, :], op=ALU.subtract)
            nc.vector.tensor_scalar_mul(out=lse[:, :], in0=lse[:, :],
                                        scalar1=float(C))
            nc.vector.tensor_tensor(out=loss_sb[:, :], in0=loss_sb[:, :],
                                    in1=lse[:, :], op=ALU.add)
        nc.sync.dma_start(out=d_loss[:].rearrange("(m p) -> p m", p=128),
                          in_=loss_sb[:, :])

        dpool.release()
        gpool.release()
        encpool.release()
        wgpool.release()
        cpool.release()

    nc.compile()
    return nc


def _prep_inputs(x_batch, context_words_batch, eps, inf_emb, W_aff, b_aff,
                 W_mu, b_mu, W_sig, b_sig, gen_sigma_emb, W_gen, b_gen,
                 with_bgen):
    f32 = lambda a: np.ascontiguousarray(np.asarray(a, dtype=np.float32))
    x_batch = np.asarray(x_batch, dtype=np.int32)
    ctx = np.asarray(context_words_batch, dtype=np.int32)
    eps = f32(eps)
    W_aff, W_mu, W_sig = f32(W_aff), f32(W_mu), f32(W_sig)
    b_aff, b_mu, b_sig = f32(b_aff), f32(b_mu), f32(b_sig)
    inf_emb, gen_sigma_emb = f32(inf_emb), f32(gen_sigma_emb)
    W_gen, b_gen = f32(W_gen), f32(b_gen)

    shared = {
        "waff1T": np.ascontiguousarray(W_aff[:, :D].T),
        "waff2T": np.ascontiguousarray(W_aff[:, D:].T),
        "wmuT": np.ascontiguousarray(W_mu.T),
        "wsigT": np.ascontiguousarray(W_sig.T),
        "baff": np.ascontiguousarray(b_aff.reshape(D, 1)),
        "bmu": np.ascontiguousarray(b_mu.reshape(D, 1)),
        "bsig": np.ascontiguousarray(b_sig.reshape(D, 1)),
        "inf_emb": inf_emb,
        "gsig_emb": gen_sigma_emb,
        "wg_aug": np.ascontiguousarray(
            np.concatenate([W_gen, b_gen.reshape(N, 1)], axis=1)),
        "wgT": np.ascontiguousarray(W_gen.T.astype(ml_dtypes.bfloat16)),
        "ident": np.eye(128, dtype=np.float32),
    }
    if with_bgen:
        NCH = (N + CHUNK - 1) // CHUNK
        bg = np.zeros((NCH * CHUNK,), dtype=ml_dtypes.bfloat16)
        bg[:N] = b_gen.astype(ml_dtypes.bfloat16)
        shared["bgen2d"] = bg.reshape(NCH, CHUNK)

    in_maps = []
    for s in range(NCORES):
        lo, hi = s * BS, (s + 1) * BS
        csh = ctx[lo:hi]                      # [BS, C]
        m = dict(shared)
        m["ctx_idx"] = np.ascontiguousarray(
            csh.reshape(NFLAT).reshape(NT, 128).T)
        m["ctx_byc"] = np.ascontiguousarray(
            np.concatenate([csh[k * 128:(k + 1) * 128, :] for k in range(M)],
                           axis=1))
        m["x_idx"] = np.ascontiguousarray(x_batch[lo:hi].reshape(M, 128).T)
        m["epsT"] = np.ascontiguousarray(eps[lo:hi].T)
        in_maps.append(m)
    return in_maps


def kernel(x_batch, context_words_batch, eps, inf_emb, W_aff, b_aff,
           W_mu, b_mu, W_sig, b_sig, gen_sigma_emb, W_gen, b_gen,
           trace=False):
    with_bgen = bool(np.any(np.asarray(b_gen) != 0))
    if with_bgen not in _CACHE:
        _CACHE[with_bgen] = _build(with_bgen)
    nc = _CACHE[with_bgen]

    in_maps = _prep_inputs(x_batch, context_words_batch, eps, inf_emb, W_aff,
                           b_aff, W_mu, b_mu, W_sig, b_sig, gen_sigma_emb,
                           W_gen, b_gen, with_bgen)
    res = run_bass_kernel_spmd(nc, in_maps, core_ids=list(range(NCORES)),
                               trace=trace)
    parts = [res.results[s]["loss_part"] for s in range(NCORES)]
    loss = np.concatenate(parts).astype(np.float64).mean()
    out = np.float32(loss)
    if trace:
        kernel.last_results = res
    return out

